# revision 1
# baseline (speedup 1.0000x reference)
"""Trainium2 Bass kernel for nn_MultiHeadSelfAttentionBlock.

Strategy (data-parallel over batch, B=32 -> 4 per core on 8 cores):
  - BN folded to per-channel scale/shift, written into a zero-padded [C,34,34]
    SBUF buffer (borders = conv padding).
  - q 1x1 conv as 40 accumulating matmuls; results stored c-major:
    qbuf[s%128, c*8 + t] so each head's Q^T tile is one contiguous slice
    (the torch .view head-split bug resolves to l = 16*c + 2*t + par, k = s_lo).
  - depthwise 3x3/s2 conv + BN + 1x1 proj: im2col tap windows staged once per
    channel chunk (shared by k and v), then 9 taps x 5 chunks of accumulating
    matmuls with weights W_tapT[c,kd] = wT[c,kd]*bnscale[c]*dw[c,tap]
    (k-side prescaled by 1/sqrt(64)); BN shift folded into a constant.
    k and v share one PSUM tile via PE column groups (0,0) / (0,64).
  - logits computed transposed [p, l] (lhsT = kf duplicated into both
    partition halves so K=64 matmul pairs pack into PE row groups 0/64);
    softmax denominator comes free as row 64 of the o-matmul by appending a
    ones column to V^T.
  - o = V'^T @ exp(logitsT) in PSUM [65, l]; denominator rows extracted by
    ACT into partitions 0/32, reciprocal via ACT ln -> exp(-x), broadcast
    across partitions via a DRAM bounce, and the normalize multiply fused
    into the PSUM->SBUF move (DVE STT), writing o_resh[(n,vd), s] directly
    via a scatter AP.
  - output proj as accumulating matmuls; layer-scale applied via a
    precomputed [128,1024] mask during the PSUM->SBUF move; residual added on
    GPSIMD; fp32 everywhere, matmuls use float32r (fast fp32 weight path).
"""

from contextlib import ExitStack

import os

import numpy as np

import concourse.bacc as bacc
import concourse.bass as bass
import concourse.tile as tile
from concourse import mybir
from concourse.masks import make_identity
from concourse.dve_ops import RECIPROCAL_APPROX_FAST, RECIP_APPROX_FAST_CONSTS

F32 = mybir.dt.float32
F32R = mybir.dt.float32r
ALU = mybir.AluOpType
ACTF = mybir.ActivationFunctionType

B, C, H, W = 32, 640, 32, 32
NH, KD, VD = 8, 64, 64
S = H * W            # 1024
P = 256              # key/value positions (16x16)
EPS = 1e-3
N_CORES = 8
BPC = B // N_CORES   # 4 batch items per core
NCH = C // 128       # 5 channel chunks


def _r(ap):
    return ap.bitcast(F32R)


def _fap(base, free_off, dims):
    """AP with base's partition dim and explicit free dims [[step, count],...]."""
    return bass.AP(tensor=base.tensor, offset=base.offset + free_off,
                   ap=[base.ap[0]] + dims)


def build_nc():
    nc = bacc.Bacc(None, target_bir_lowering=False, debug=False)

    din = {}
    def dt_in(name, shape):
        din[name] = nc.dram_tensor(name, shape, F32, kind="ExternalInput")
        return din[name]

    x4 = dt_in("x", [BPC, C, H, W])
    q_w = dt_in("q_w", [NH * KD, C])
    k_w = dt_in("k_w", [KD, C])
    v_w = dt_in("v_w", [VD, C])
    out_w = dt_in("out_w", [C, NH * VD])
    k_dw = dt_in("k_dw_w", [C, 1, 3, 3])
    v_dw = dt_in("v_dw_w", [C, 1, 3, 3])
    for p in ("in", "k", "v"):
        for s in ("gamma", "beta", "mean", "var"):
            dt_in(f"{p}_bn_{s}", [C])
    ls = dt_in("ls_gamma", [W])
    out4 = nc.dram_tensor("out", [BPC, C, H, W], F32, kind="ExternalOutput")
    KSTAGE = int(os.environ.get("KSTAGE", "99"))

    with tile.TileContext(nc) as tc, ExitStack() as ctx:
        wp = ctx.enter_context(tc.tile_pool(name="wp", bufs=1))
        stg = ctx.enter_context(tc.tile_pool(name="stg", bufs=2))
        # PSUM pools (bank-granular): mm 2 + lg 2 + op 4 = 8 banks
        mmp = ctx.enter_context(tc.tile_pool(name="mmp", bufs=2, space="PSUM"))
        lgp = ctx.enter_context(tc.tile_pool(name="lgp", bufs=2, space="PSUM"))
        opp = ctx.enter_context(tc.tile_pool(name="opp", bufs=4, space="PSUM"))
        # SBUF working pools
        xin = ctx.enter_context(tc.tile_pool(name="xin", bufs=2))
        xres = ctx.enter_context(tc.tile_pool(name="xres", bufs=2))
        xnp = ctx.enter_context(tc.tile_pool(name="xnp", bufs=NCH))
        xcp = ctx.enter_context(tc.tile_pool(name="xcp", bufs=1))
        qbp = ctx.enter_context(tc.tile_pool(name="qbp", bufs=2))
        ep = ctx.enter_context(tc.tile_pool(name="ep", bufs=3))
        dal = ctx.enter_context(tc.tile_pool(name="dal", bufs=1))
        rbcp = ctx.enter_context(tc.tile_pool(name="rbcp", bufs=2))
        orp = ctx.enter_context(tc.tile_pool(name="orp", bufs=4))
        osb = ctx.enter_context(tc.tile_pool(name="osb", bufs=2))
        kvp = ctx.enter_context(tc.tile_pool(name="kvp", bufs=2))
        drp = ctx.enter_context(tc.tile_pool(name="drp", bufs=4, space="DRAM"))

        # ---------------- setup: identity ----------------
        ident = wp.tile([128, 128], F32, tag="ident", name="ident")
        make_identity(nc, ident[:])

        def pe_transpose(dst_sbuf_ap, src_sbuf_ap, scale=1.0, rnd=False):
            """dst[f, p] = src[p, f] via PE; src [p, f] with p,f <= 128."""
            pdim = src_sbuf_ap.shape[0]
            fdim = src_sbuf_ap.free_size()
            tp = mmp.tile([128, 512], F32, tag="mm", name="tp")
            nc.tensor.transpose(tp[:fdim, :pdim], src_sbuf_ap,
                                ident[:pdim, :pdim])
            dst = _r(dst_sbuf_ap) if rnd else dst_sbuf_ap
            nc.scalar.activation(dst, tp[:fdim, :pdim], ACTF.Copy,
                                 scale=scale)

        # ---------------- setup: BN scale/shift ----------------
        eps_t = wp.tile([128, 1], F32, tag="eps", name="eps")
        nc.gpsimd.memset(eps_t[:], EPS)
        bnss = {}  # (prefix, ch) -> (scale [128,1], shift [128,1])
        for pfx in ("in", "k", "v"):
            for ch in range(NCH):
                g = stg.tile([128, 1], F32, tag="bnl0", name="bnl0")
                be = stg.tile([128, 1], F32, tag="bnl1", name="bnl1")
                m = stg.tile([128, 1], F32, tag="bnl2", name="bnl2")
                v = stg.tile([128, 1], F32, tag="bnl3", name="bnl3")
                cs = slice(128 * ch, 128 * (ch + 1))
                nc.sync.dma_start(out=g[:], in_=din[f"{pfx}_bn_gamma"][cs].unsqueeze(1))
                nc.sync.dma_start(out=be[:], in_=din[f"{pfx}_bn_beta"][cs].unsqueeze(1))
                nc.sync.dma_start(out=m[:], in_=din[f"{pfx}_bn_mean"][cs].unsqueeze(1))
                nc.sync.dma_start(out=v[:], in_=din[f"{pfx}_bn_var"][cs].unsqueeze(1))
                sc = wp.tile([128, 1], F32, tag=f"sc_{pfx}{ch}", name=f"sc_{pfx}{ch}")
                sh = wp.tile([128, 1], F32, tag=f"sh_{pfx}{ch}", name=f"sh_{pfx}{ch}")
                nc.scalar.activation(sc[:], v[:], ACTF.Sqrt, bias=eps_t[:])
                nc.vector.reciprocal(sc[:], sc[:])
                nc.vector.tensor_mul(sc[:], sc[:], g[:])
                nc.vector.tensor_mul(sh[:], m[:], sc[:])
                nc.vector.tensor_sub(sh[:], be[:], sh[:])
                bnss[(pfx, ch)] = (sc, sh)

        # ---------------- setup: transposed weights ----------------
        q_wT = [wp.tile([128, 512], F32, tag=f"qwT{j}", name=f"qwT{j}")
                for j in range(NCH)]
        for i in range(4):
            st = stg.tile([128, 640], F32, tag="wstage", name="wstage")
            nc.sync.dma_start(out=st[:], in_=q_w[128 * i:128 * (i + 1), :])
            for j in range(NCH):
                pe_transpose(q_wT[j][:, 128 * i:128 * (i + 1)],
                             st[:, 128 * j:128 * (j + 1)], rnd=True)

        out_wT = [wp.tile([128, 640], F32, tag=f"owT{j}", name=f"owT{j}")
                  for j in range(4)]
        for i in range(NCH):
            st = stg.tile([128, 512], F32, tag="wstage", name="wstage")
            nc.sync.dma_start(out=st[:], in_=out_w[128 * i:128 * (i + 1), :])
            for j in range(4):
                pe_transpose(out_wT[j][:, 128 * i:128 * (i + 1)],
                             st[:, 128 * j:128 * (j + 1)], rnd=True)

        kv_wT = {}
        for nm, wdr, scl in (("k", k_w, 0.125), ("v", v_w, 1.0)):
            st = stg.tile([64, 640], F32, tag="kvstage", name="kvstage")
            nc.sync.dma_start(out=st[:], in_=wdr[:, :])
            for j in range(NCH):
                wt = wp.tile([128, 64], F32, tag=f"{nm}wT{j}", name=f"{nm}wT{j}")
                pe_transpose(wt[:], st[:, 128 * j:128 * (j + 1)], scale=scl)
                kv_wT[(nm, j)] = wt

        # ---------------- setup: conv tap weights + consts ----------------
        wtap = {}
        kv_const = {}
        for nm, dwdr in (("k", k_dw), ("v", v_dw)):
            cps = mmp.tile([64, 512], F32, tag="mm", name="cps")
            for ch in range(NCH):
                dw = stg.tile([128, 9], F32, tag="dwl", name="dwl")
                nc.sync.dma_start(
                    out=dw[:],
                    in_=dwdr[128 * ch:128 * (ch + 1), 0, :, :].rearrange(
                        "c a b -> c (a b)"))
                sc, sh = bnss[(nm, ch)]
                s9 = stg.tile([128, 9], F32, tag="s9", name="s9")
                nc.vector.tensor_scalar_mul(s9[:], dw[:], sc[:])
                for t in range(9):
                    wtt = wp.tile([128, 64], F32, tag=f"wtap_{nm}{ch}_{t}",
                                  name=f"wtap_{nm}{ch}_{t}")
                    nc.vector.tensor_scalar_mul(_r(wtt[:]),
                                                kv_wT[(nm, ch)][:],
                                                s9[:, t:t + 1])
                    wtap[(nm, ch, t)] = wtt
                nc.tensor.matmul(cps[:64, 0:1], kv_wT[(nm, ch)][:], sh[:],
                                 start=(ch == 0), stop=(ch == NCH - 1))
            cst = wp.tile([64, 1], F32, tag=f"const_{nm}", name=f"const_{nm}")
            nc.scalar.activation(cst[:], cps[:64, 0:1], ACTF.Copy)
            kv_const[nm] = cst

        # ---------------- setup: zero/one consts ----------------
        zeros16 = wp.tile([128, 16], F32, tag="zeros16", name="zeros16")
        nc.gpsimd.memset(zeros16[:], 0.0)
        ones1 = wp.tile([128, 1], F32, tag="ones1", name="ones1")
        nc.gpsimd.memset(ones1[:], 1.0)

        # ---------------- setup: layer-scale mask ----------------
        lsmask = wp.tile([128, 1024], F32, tag="lsmask", name="lsmask")
        ls_b = bass.AP(tensor=ls, offset=0, ap=[[0, 128], [1, 32]])
        for rr in range(32):
            nc.sync.dma_start(out=lsmask[:, 32 * rr:32 * (rr + 1)], in_=ls_b)

        # ================= per batch item =================
        im2col_eng = [nc.vector, nc.gpsimd, nc.scalar]
        for b in range(BPC):
            # ---- load x, BN into flat xn buffer ----
            xns = []
            for ch in range(NCH):
                xt = xin.tile([128, 1024], F32, tag="xin", name="xin")
                nc.sync.dma_start(
                    out=xt[:],
                    in_=x4[b, 128 * ch:128 * (ch + 1), :, :].rearrange(
                        "c h w -> c (h w)"))
                xn = xnp.tile([128, 1024], F32, tag="xn", name="xn")
                sc, sh = bnss[("in", ch)]
                nc.gpsimd.tensor_scalar(
                    out=_r(xn[:]), in0=xt[:],
                    scalar1=sc[:], scalar2=sh[:], op0=ALU.mult, op1=ALU.add)
                xns.append(xn)

            # ---- q projection -> qbuf [s%128, c*8 + t] (c-major) ----
            qbuf = qbp.tile([128, 4096], F32, tag="qbuf", name="qbuf")
            for t in range(8):
                qp = mmp.tile([128, 512], F32, tag="mm", name="qp")
                for ch in range(NCH):
                    lhsT = xns[ch][:, 128 * t:128 * (t + 1)]
                    nc.tensor.matmul(qp[:], _r(lhsT), _r(q_wT[ch][:]),
                                     start=(ch == 0), stop=(ch == NCH - 1))
                nc.vector.tensor_copy(_r(_fap(qbuf[:], t, [[8, 512]])), qp[:])

            if KSTAGE == 1:
                nc.sync.dma_start(
                    out=out4[b, 0:128, :, :].rearrange("c h w -> c (h w)"),
                    in_=qbuf[:, 0:1024])
                continue
            # ---- im2col + dw-conv + BN + 1x1 proj for k and v ----
            kfp = mmp.tile([64, 256], F32, tag="mm", name="kfp")
            vfp = mmp.tile([64, 256], F32, tag="mm", name="vfp")
            for ch in range(NCH):
                xc = xcp.tile([128, 9 * 256], F32, tag="xcol", name="xcol")
                xnv = xns[ch][:].rearrange("p (a b) -> p a b", a=32)
                for t in range(9):
                    dy, dx = t // 3, t % 3
                    oh0 = 1 if dy == 0 else 0
                    ow0 = 1 if dx == 0 else 0
                    if oh0:
                        nc.vector.tensor_copy(_r(xc[:, 256 * t:256 * t + 16]),
                                              zeros16[:])
                    if ow0:
                        nc.vector.tensor_copy(
                            _r(_fap(xc[:], 256 * t, [[16, 16], [1, 1]])),
                            zeros16[:])
                    r0 = 2 * oh0 + dy - 1
                    c0 = 2 * ow0 + dx - 1
                    srcap = xnv[:, r0:r0 + 2 * (16 - oh0) - 1:2,
                                c0:c0 + 2 * (16 - ow0) - 1:2]
                    dst2 = _r(_fap(xc[:], 256 * t + 16 * oh0 + ow0,
                                   [[16, 16 - oh0], [1, 16 - ow0]]))
                    if t % 3 == 2:
                        nc.scalar.activation(dst2, srcap, ACTF.Copy)
                    else:
                        im2col_eng[t % 3].tensor_copy(dst2, srcap)
                for t in range(9):
                    first = (ch == 0 and t == 0)
                    last = (ch == NCH - 1 and t == 8)
                    xslice = xc[:, 256 * t:256 * (t + 1)]
                    nc.tensor.matmul(
                        kfp[:], _r(wtap[("k", ch, t)][:]), _r(xslice),
                        start=first, stop=last)
                    nc.tensor.matmul(
                        vfp[:], _r(wtap[("v", ch, t)][:]), _r(xslice),
                        start=first, stop=last)
            # kf duplicated into both halves (base-partition match for logits)
            kfdup = kvp.tile([128, 256], F32, tag="f_k", name="f_k")
            nc.vector.tensor_scalar_add(_r(kfdup[0:64, :]), kfp[:],
                                        kv_const["k"][:])
            nc.vector.tensor_scalar_add(_r(kfdup[64:128, :]), kfp[:],
                                        kv_const["k"][:])
            vf = kvp.tile([64, 256], F32, tag="f_v", name="f_v")
            nc.vector.tensor_scalar_add(vf[:], vfp[:],
                                        kv_const["v"][:])

            # V' = vf^T with ones column: 2 tiles [128, 65]
            vT = []
            for pt in range(2):
                vpt = kvp.tile([128, 65], F32, tag=f"vT{pt}", name=f"vT{pt}")
                pe_transpose(vpt[:, 0:64], vf[:, 128 * pt:128 * (pt + 1)],
                             rnd=True)
                nc.vector.tensor_copy(_r(vpt[:, 64:65]), ones1[:])
                vT.append(vpt)

            if KSTAGE == 2:
                nc.sync.dma_start(
                    out=out4[b, 0:128, 0:8, :].rearrange("c h w -> c (h w)"),
                    in_=kfdup[:, :])
                nc.sync.dma_start(
                    out=out4[b, 128:256, 0:2, :].rearrange("c h w -> c (h w)"),
                    in_=vT[0][:, 0:64])
                continue
            o_resh = [orp.tile([128, 1024], F32, tag="oresh", name="oresh")
                      for _ in range(4)]

            # ---- attention heads (pairs share a reciprocal) ----
            for pair in range(4):
                dall = dal.tile([33, 1024], F32, tag="dall", name="dall")
                ops_pair = []
                for n in (2 * pair, 2 * pair + 1):
                    E = [ep.tile([128, 1024], F32, tag="E", name="E")
                         for _ in range(2)]
                    for pt in range(2):
                        for par in range(2):
                            lg = lgp.tile([128, 512], F32, tag="lg", name="lg")
                            rhs = qbuf[64 * par:64 * (par + 1),
                                       512 * n:512 * (n + 1)]
                            nc.tensor.matmul(
                                lg[:],
                                _r(kfdup[64 * par:64 * (par + 1),
                                         128 * pt:128 * (pt + 1)]),
                                _r(rhs), start=True, stop=True)
                            nc.scalar.activation(
                                _r(E[pt][:, 512 * par:512 * (par + 1)]), lg[:],
                                ACTF.Exp)
                    o_ps = []
                    for par in range(2):
                        op_t = opp.tile([65, 512], F32, tag="op", name="op")
                        for pt in range(2):
                            nc.tensor.matmul(
                                op_t[:], _r(vT[pt][:]),
                                _r(E[pt][:, 512 * par:512 * (par + 1)]),
                                start=(pt == 0), stop=(pt == 1))
                        nc.vector.tensor_copy(
                            dall[32 * (n % 2):32 * (n % 2) + 1,
                                 512 * par:512 * (par + 1)],
                            op_t[64:65, :])
                        o_ps.append(op_t)
                    ops_pair.append((n, o_ps))

                rec = dal.tile([33, 1024], F32, tag="rec", name="rec")
                nc.vector._custom_dve(
                    RECIPROCAL_APPROX_FAST, out=rec[:], in0=dall[:],
                    s0=RECIP_APPROX_FAST_CONSTS["s0"],
                    s1=RECIP_APPROX_FAST_CONSTS["s1"],
                    imm2=RECIP_APPROX_FAST_CONSTS["imm2"])

                for n, o_ps in ops_pair:
                    dsc = drp.tile([1, 1024], F32, tag="dscr", name="dscr")
                    nc.sync.dma_start(
                        out=dsc[:], in_=rec[32 * (n % 2):32 * (n % 2) + 1, :])
                    rbc = rbcp.tile([64, 1024], F32, tag="rbc", name="rbc")
                    nc.sync.dma_start(
                        out=rbc[:],
                        in_=bass.AP(tensor=dsc.tensor, offset=dsc.offset,
                                    ap=[[0, 64], [1, 1024]]))
                    dst = o_resh[n // 2]
                    for par in range(2):
                        # scatter: col = 16*c + 2*t + par, iteration c-major
                        out_ap = _fap(dst[64 * (n % 2):64 * (n % 2) + 64], par,
                                      [[16, 64], [2, 8]])
                        nc.vector.scalar_tensor_tensor(
                            out=_r(out_ap), in0=o_ps[par][0:64, :], scalar=1.0,
                            in1=rbc[:, 512 * par:512 * (par + 1)],
                            op0=ALU.mult, op1=ALU.mult)

            if KSTAGE == 3:
                for c2 in range(4):
                    nc.sync.dma_start(
                        out=out4[b, 128 * c2:128 * (c2 + 1), :, :].rearrange(
                            "c h w -> c (h w)"),
                        in_=o_resh[c2][:, :])
                continue
            # ---- output projection + layer scale + residual ----
            for ch in range(NCH):
                xr = xres.tile([128, 1024], F32, tag="xres", name="xres")
                nc.sync.dma_start(
                    out=xr[:],
                    in_=x4[b, 128 * ch:128 * (ch + 1), :, :].rearrange(
                        "c h w -> c (h w)"))
                ot = osb.tile([128, 1024], F32, tag="outsb", name="outsb")
                for shalf in range(2):
                    po = mmp.tile([128, 512], F32, tag="mm", name="po")
                    for nv in range(4):
                        nc.tensor.matmul(
                            po[:],
                            _r(out_wT[nv][:, 128 * ch:128 * (ch + 1)]),
                            _r(o_resh[nv][:, 512 * shalf:512 * (shalf + 1)]),
                            start=(nv == 0), stop=(nv == 3))
                    sl = slice(512 * shalf, 512 * (shalf + 1))
                    nc.vector.scalar_tensor_tensor(
                        out=ot[:, sl], in0=po[:], scalar=1.0,
                        in1=lsmask[:, sl], op0=ALU.mult, op1=ALU.mult)
                    nc.gpsimd.tensor_tensor(
                        out=ot[:, sl], in0=ot[:, sl], in1=xr[:, sl], op=ALU.add)
                nc.sync.dma_start(
                    out=out4[b, 128 * ch:128 * (ch + 1), :, :].rearrange(
                        "c h w -> c (h w)"),
                    in_=ot[:])

    nc.finalize()
    return nc


_NC_CACHE = None


def kernel(**inputs):
    global _NC_CACHE
    from concourse.bass_utils import run_bass_kernel_spmd

    if _NC_CACHE is None:
        _NC_CACHE = build_nc()
    nc = _NC_CACHE

    x = np.ascontiguousarray(np.asarray(inputs["x"], dtype=np.float32))
    wnames = ["q_w", "k_w", "v_w", "out_w", "k_dw_w", "v_dw_w", "ls_gamma"] + \
        [f"{p}_bn_{s}" for p in ("in", "k", "v")
         for s in ("gamma", "beta", "mean", "var")]
    base = {n: np.ascontiguousarray(np.asarray(inputs[n], dtype=np.float32))
            for n in wnames}
    in_maps = []
    for c in range(N_CORES):
        m = dict(base)
        m["x"] = x[c * BPC:(c + 1) * BPC]
        in_maps.append(m)

    res = run_bass_kernel_spmd(nc, in_maps, core_ids=list(range(N_CORES)))
    out = np.concatenate([res.results[c]["out"] for c in range(N_CORES)], axis=0)
    return out.astype(np.float32)



# revision 21
# speedup vs baseline: 1.7068x; 1.7068x over previous
"""Trainium2 Bass kernel for nn_MultiHeadSelfAttentionBlock.

Data-parallel over batch (B=32 -> 4 per core on 8 cores). Single-core
pipeline, bf16 matmul operands (fp32 PSUM accumulation) throughout:

  - All weight preprocessing happens on host (numpy): BN folded to
    per-channel scale/shift, q/out projections transposed into lhsT
    layout, k/v 1x1 projections merged with the depthwise-conv taps and
    k/v BN into 45 per-(chunk,tap) [128c, 64kd_k||64kd_v] bf16 blocks
    (k-side prescaled by 1/sqrt(64)), BN-shift constants reduced to a
    [128,1] vector, layer-scale replicated to a [8,1024] row table.
  - Per item: x loaded once (kept fp32 for the residual); BN applied on
    GPSIMD writing bf16 into a zero-padded [c, 34x34] buffer.  q proj
    reads 128-pixel slices of the padded buffer as the stationary
    operand; conv taps read strided 16x16/stride-2 windows as the
    moving operand -- no im2col staging, and k+v share each matmul.
  - Logits computed transposed [p, l] per head (the torch .view
    head-split bug resolves to l = 16*c + 2*t + par, kd = s_lo); the
    softmax denominator comes free as row 64 of the o-matmul via a ones
    column appended to V^T.  exp on Scalar (bf16 out), denominator rows
    copied to SBUF on Scalar, one reciprocal per item on DVE with
    layer-scale folded in, broadcast across partitions via a DRAM
    bounce per head-pair, normalize on GPSIMD.
  - Output projection accumulates in PSUM; epilogue is a single DVE add
    of the fp32 residual.  Emission is software-pipelined: item b-1's
    output projection is emitted between taps(b) and attention(b) so
    the PE never idles on the denominator DRAM round-trip.
"""

from contextlib import ExitStack

import os

import numpy as np

import concourse.bacc as bacc
import concourse.bass as bass
import concourse.tile as tile
from concourse import mybir
from concourse.masks import make_identity
from concourse.dve_ops import RECIPROCAL_APPROX_FAST, RECIP_APPROX_FAST_CONSTS

F32 = mybir.dt.float32
BF16 = mybir.dt.bfloat16
ALU = mybir.AluOpType
ACTF = mybir.ActivationFunctionType

B, C, H, W = 32, 640, 32, 32
NH, KD, VD = 8, 64, 64
S = H * W            # 1024
P = 256              # key/value positions (16x16)
EPS = 1e-3
N_CORES = 8
BPC = B // N_CORES   # 4 batch items per core
NCH = C // 128       # 5 channel chunks
PW = 34              # padded image width
PSZ = PW * PW        # 1156


def _fap(base, free_off, dims):
    """AP with base's partition dim and explicit free dims [[step, count],...]."""
    return bass.AP(tensor=base.tensor, offset=base.offset + free_off,
                   ap=[base.ap[0]] + dims)


def build_nc():
    nc = bacc.Bacc(None, target_bir_lowering=False, debug=False)

    x4 = nc.dram_tensor("x", [BPC, C, H, W], F32, kind="ExternalInput")
    qwT_d = nc.dram_tensor("p_qwT", [128, NCH * 512], BF16, kind="ExternalInput")
    wtap_d = nc.dram_tensor("p_wtap", [128, NCH * 9 * 128], BF16,
                            kind="ExternalInput")
    owT_d = nc.dram_tensor("p_owT", [128, 4 * C], BF16, kind="ExternalInput")
    kvc_d = nc.dram_tensor("p_kvconst", [128, 1], F32, kind="ExternalInput")
    bnio_d = nc.dram_tensor("p_bnio", [128, 2 * NCH], F32, kind="ExternalInput")
    lsr_d = nc.dram_tensor("p_lsrow", [128, 2 * S], BF16, kind="ExternalInput")
    out4 = nc.dram_tensor("out", [BPC, C, H, W], F32, kind="ExternalOutput")
    KSTAGE = int(os.environ.get("KSTAGE", "99"))

    with tile.TileContext(nc) as tc, ExitStack() as ctx:
        wp = ctx.enter_context(tc.tile_pool(name="wp", bufs=1))
        # single PSUM pool, tags sized to exactly 8 banks:
        #   mm 2x[128,512]f32 + kvf 1x[128,256]f32 + lg 2x[128,512]f32
        #   + op 3x[65,512]f32
        pp = ctx.enter_context(tc.tile_pool(name="pp", bufs=1, space="PSUM"))
        xin = ctx.enter_context(tc.tile_pool(name="xin", bufs=2 * NCH))
        xnfp = ctx.enter_context(tc.tile_pool(name="xnfp", bufs=NCH))
        qbp = ctx.enter_context(tc.tile_pool(name="qbp", bufs=2))
        ep = ctx.enter_context(tc.tile_pool(name="ep", bufs=4))
        kvp = ctx.enter_context(tc.tile_pool(name="kvp", bufs=2))
        orp = ctx.enter_context(tc.tile_pool(name="orp", bufs=8))
        onp = ctx.enter_context(tc.tile_pool(name="onp", bufs=4))
        rbcp = ctx.enter_context(tc.tile_pool(name="rbcp", bufs=4))
        dap = ctx.enter_context(tc.tile_pool(name="dap", bufs=2))
        osb = ctx.enter_context(tc.tile_pool(name="osb", bufs=2))
        drp = ctx.enter_context(tc.tile_pool(name="drp", bufs=2, space="DRAM"))

        # ---------------- setup ----------------
        identf = wp.tile([64, 64], F32, tag="identf", name="identf")
        make_identity(nc, identf[:])
        ones1 = wp.tile([128, 1], BF16, tag="ones1", name="ones1")
        nc.gpsimd.memset(ones1[:], 1.0)

        qwT = wp.tile([128, NCH * 512], BF16, tag="qwT", name="qwT")
        nc.sync.dma_start(out=qwT[:], in_=qwT_d[:, :])
        wtap = wp.tile([128, NCH * 9 * 128], BF16, tag="wtap", name="wtap")
        nc.sync.dma_start(out=wtap[:], in_=wtap_d[:, :])
        owT = wp.tile([128, 4 * C], BF16, tag="owT", name="owT")
        nc.sync.dma_start(out=owT[:], in_=owT_d[:, :])
        kvc = wp.tile([128, 1], F32, tag="kvc", name="kvc")
        nc.sync.dma_start(out=kvc[:], in_=kvc_d[:, :])
        bnio = wp.tile([128, 2 * NCH], F32, tag="bnio", name="bnio")
        nc.sync.dma_start(out=bnio[:], in_=bnio_d[:, :])
        lsrow = wp.tile([128, 2 * S], BF16, tag="lsrow", name="lsrow")
        nc.sync.dma_start(out=lsrow[:], in_=lsr_d[:, :])

        # denominator staging: head n lives at partition 32*(n%4), column
        # block S*(n//4) (engines only address start partitions 0/32/64/96).
        dall_t = [dap.tile([128, 2 * S], F32, tag="dall", name="dall")
                  for _ in range(2)]
        for i in range(2):
            nc.gpsimd.memset(dall_t[i][:], 1.0)

        # zero-padded xn buffers: 2 item-slots x NCH chunks; borders are
        # zeroed once here and only the 32x32 interior is rewritten per item.
        xnpad = [[wp.tile([128, PSZ], BF16, tag=f"xnp{i}_{ch}",
                          name=f"xnp{i}_{ch}")
                  for ch in range(NCH)] for i in range(2)]
        for i in range(2):
            for ch in range(NCH):
                nc.gpsimd.memset(xnpad[i][ch][:], 0.0)

        def xn_interior(t):
            """interior write AP: [128, 32, 32] at offset (1,1) of 34x34."""
            return _fap(t, PW + 1, [[PW, H], [1, W]])

        def xn_tap(t, dy, dx):
            """moving conv-tap window: stride-2 16x16 -> [128c, 256p]."""
            return _fap(t, PW * dy + dx, [[2 * PW, 16], [2, 16]])

        prev = None  # (b, o_norm tiles, x tiles)

        def emit_outproj(bp, onorm_p, xt_p):
            for ch in range(NCH):
                ot = osb.tile([128, S], F32, tag="outsb", name="outsb")
                for sh in range(2):
                    po = pp.tile([128, 512], F32, tag="mm", bufs=2, name="po")
                    for nv in range(4):
                        nc.tensor.matmul(
                            po[:], owT[:, C * nv + 128 * ch:C * nv + 128 * (ch + 1)],
                            onorm_p[nv][:, 512 * sh:512 * (sh + 1)],
                            start=(nv == 0), stop=(nv == 3))
                    sl = slice(512 * sh, 512 * (sh + 1))
                    nc.vector.tensor_tensor(out=ot[:, sl], in0=po[:],
                                            in1=xt_p[ch][:, sl], op=ALU.add)
                nc.sync.dma_start(
                    out=out4[bp, 128 * ch:128 * (ch + 1), :, :].rearrange(
                        "c h w -> c (h w)"),
                    in_=ot[:])

        # ================= per batch item =================
        for b in range(BPC):
            slot = b % 2
            # ---- load x (kept for residual), BN -> flat + padded bf16 ----
            xts, xnfs = [], []
            for ch in range(NCH):
                xt = xin.tile([128, S], F32, tag="xin", name="xin")
                nc.sync.dma_start(
                    out=xt[:],
                    in_=x4[b, 128 * ch:128 * (ch + 1), :, :].rearrange(
                        "c h w -> c (h w)"))
                xts.append(xt)
            for ch in range(NCH):
                xnf = xnfp.tile([128, S], BF16, tag="xnf", name="xnf")
                nc.gpsimd.tensor_scalar(
                    out=xnf[:], in0=xts[ch][:],
                    scalar1=bnio[:, ch:ch + 1], scalar2=bnio[:, NCH + ch:NCH + ch + 1],
                    op0=ALU.mult, op1=ALU.add)
                nc.scalar.activation(xn_interior(xnpad[slot][ch]), xnf[:],
                                     ACTF.Copy)
                xnfs.append(xnf)

            # ---- q projection -> qbuf [s%128, c*8 + t] (c-major) ----
            qbuf = qbp.tile([128, 8 * 512], BF16, tag="qbuf", name="qbuf")
            for t in range(8):
                qp = pp.tile([128, 512], F32, tag="mm", bufs=2, name="qp")
                for ch in range(NCH):
                    nc.tensor.matmul(qp[:], xnfs[ch][:, 128 * t:128 * (t + 1)],
                                     qwT[:, 512 * ch:512 * (ch + 1)],
                                     start=(ch == 0), stop=(ch == NCH - 1))
                nc.vector.tensor_copy(_fap(qbuf[:], t, [[8, 512]]), qp[:])

            if KSTAGE == 1:
                nc.sync.dma_start(
                    out=out4[b, 0:128, :, :].rearrange("c h w -> c (h w)"),
                    in_=qbuf[:, 0:1024].bitcast(F32))
                continue

            # ---- merged k|v conv taps -> kvf PSUM [64kf || 64vf, 256] ----
            kvf = pp.tile([128, 256], F32, tag="kvf", bufs=1, name="kvf")
            for ch in range(NCH):
                for t in range(9):
                    nc.tensor.matmul(
                        kvf[:],
                        wtap[:, 128 * (9 * ch + t):128 * (9 * ch + t + 1)],
                        xn_tap(xnpad[slot][ch], t // 3, t % 3),
                        start=(ch == 0 and t == 0),
                        stop=(ch == NCH - 1 and t == 8))
            kfdup = kvp.tile([128, 256], BF16, tag="f_k", name="f_k")
            nc.vector.tensor_scalar_add(kfdup[0:64, :], kvf[0:64, :],
                                        kvc[0:64, :])
            nc.vector.tensor_scalar_add(kfdup[64:128, :], kvf[0:64, :],
                                        kvc[0:64, :])
            vf = kvp.tile([64, 256], F32, tag="f_v", name="f_v")
            nc.vector.tensor_scalar_add(vf[:], kvf[64:128, :], kvc[64:128, :])

            # V' = vf^T with ones column: 2 tiles [128, 65] bf16
            vT = []
            for pt in range(2):
                tp = pp.tile([128, 512], F32, tag="mm", bufs=2, name="tp")
                nc.tensor.transpose(tp[:128, 0:64],
                                    vf[:, 128 * pt:128 * (pt + 1)],
                                    identf[0:64, 0:64])
                vpt = kvp.tile([128, 65], BF16, tag=f"vT{pt}", name=f"vT{pt}")
                nc.scalar.activation(vpt[:, 0:64], tp[:128, 0:64], ACTF.Copy)
                nc.vector.tensor_copy(vpt[:, 64:65], ones1[:])
                vT.append(vpt)

            if KSTAGE == 2:
                nc.sync.dma_start(
                    out=out4[b, 0:128, 0:4, :].rearrange("c h w -> c (h w)"),
                    in_=kfdup[:, :].bitcast(F32))
                nc.sync.dma_start(
                    out=out4[b, 128:256, 0:1, :].rearrange("c h w -> c (h w)"),
                    in_=vT[0][:, 0:64].bitcast(F32))
                continue

            # ---- previous item's output projection (pipelined) ----
            if prev is not None:
                emit_outproj(*prev)
                prev = None

            # ---- attention ----
            dall = dall_t[slot]
            o_resh = [orp.tile([128, S], BF16, tag="oresh", name="oresh")
                      for _ in range(4)]
            for n in range(NH):
                E = [ep.tile([128, S], BF16, tag="E", name="E")
                     for _ in range(2)]
                for pt in range(2):
                    for par in range(2):
                        lg = pp.tile([128, 512], F32, tag="lg", bufs=2,
                                     name="lg")
                        nc.tensor.matmul(
                            lg[:],
                            kfdup[64 * par:64 * (par + 1),
                                  128 * pt:128 * (pt + 1)],
                            qbuf[64 * par:64 * (par + 1),
                                 512 * n:512 * (n + 1)],
                            start=True, stop=True)
                        nc.scalar.activation(
                            E[pt][:, 512 * par:512 * (par + 1)], lg[:],
                            ACTF.Exp)
                for par in range(2):
                    op_t = pp.tile([65, 512], F32, tag="op", bufs=3, name="op")
                    for pt in range(2):
                        nc.tensor.matmul(
                            op_t[:], vT[pt][:],
                            E[pt][:, 512 * par:512 * (par + 1)],
                            start=(pt == 0), stop=(pt == 1))
                    doff = S * (n // 4) + 512 * par
                    nc.scalar.activation(
                        dall[32 * (n % 4):32 * (n % 4) + 1, doff:doff + 512],
                        op_t[64:65, :], ACTF.Copy)
                    # unnormalized scatter: col = 16*c + 2*t + par
                    nc.vector.tensor_copy(
                        _fap(o_resh[n // 2][64 * (n % 2):64 * (n % 2) + 64],
                             par, [[16, 64], [2, 8]]),
                        op_t[0:64, :])

            # ---- denominator: reciprocal (+ layer scale), bcast, normalize
            rec = dap.tile([128, 2 * S], F32, tag="rec", bufs=1, name="rec")
            nc.vector._custom_dve(
                RECIPROCAL_APPROX_FAST, out=rec[:], in0=dall[:],
                s0=RECIP_APPROX_FAST_CONSTS["s0"],
                s1=RECIP_APPROX_FAST_CONSTS["s1"],
                imm2=RECIP_APPROX_FAST_CONSTS["imm2"])
            dall2 = dap.tile([128, 2 * S], BF16, tag="dall2", name="dall2")
            nc.vector.tensor_tensor(out=dall2[:], in0=rec[:], in1=lsrow[:],
                                    op=ALU.mult)
            # DRAM rows come out in n%4-major order: head n -> row 2*(n%4)+n//4
            dscr = drp.tile([4, 2 * S], BF16, tag="dscr", name="dscr")
            for m in range(4):
                nc.sync.dma_start(out=dscr[m:m + 1, :],
                                  in_=dall2[32 * m:32 * m + 1, :])
            o_norm = []
            rbcs = []
            for c2 in range(4):
                r0 = 2 * ((2 * c2) % 4) + (2 * c2) // 4
                rbc = rbcp.tile([128, S], BF16, tag="rbc", name="rbc")
                rbcs.append(rbc)
                nc.sync.dma_start(
                    out=rbc[:],
                    in_=bass.AP(tensor=dscr.tensor,
                                offset=dscr.offset + S * r0,
                                ap=[[2 * S, 2], [0, 64], [1, S]]))
                on = onp.tile([128, S], BF16, tag="onorm", name="onorm")
                nc.gpsimd.tensor_tensor(out=on[:], in0=o_resh[c2][:],
                                        in1=rbc[:], op=ALU.mult)
                o_norm.append(on)

            if KSTAGE == 4:
                nc.sync.dma_start(
                    out=out4[b, 0:128, 0:16, :].rearrange("c h w -> c (h w)"),
                    in_=dall[:, 0:512])
                nc.sync.dma_start(
                    out=out4[b, 0:128, 16:32, :].rearrange("c h w -> c (h w)"),
                    in_=dall[:, 1024:1536])
                nc.sync.dma_start(
                    out=out4[b, 128:256, 0:16, :].rearrange("c h w -> c (h w)"),
                    in_=rec[:, 0:512])
                nc.sync.dma_start(
                    out=out4[b, 128:256, 16:32, :].rearrange("c h w -> c (h w)"),
                    in_=rec[:, 1024:1536])
                nc.sync.dma_start(
                    out=out4[b, 256:384, 0:16, :].rearrange("c h w -> c (h w)"),
                    in_=o_resh[1][:, :].bitcast(F32))
                nc.sync.dma_start(
                    out=out4[b, 384:512, 0:32, :].rearrange("c h w -> c (h w)"),
                    in_=dall2[:, :].bitcast(F32))
                nc.sync.dma_start(
                    out=out4[b, 512:640, 0:16, :].rearrange("c h w -> c (h w)"),
                    in_=rbcs[1][:, :].bitcast(F32))
                continue
            if KSTAGE == 3:
                for c2 in range(4):
                    nc.sync.dma_start(
                        out=out4[b, 128 * c2:128 * (c2 + 1), 0:16, :].rearrange(
                            "c h w -> c (h w)"),
                        in_=o_norm[c2][:, :].bitcast(F32))
                continue
            prev = (b, o_norm, xts)

        if prev is not None:
            emit_outproj(*prev)

    nc.finalize()
    return nc


def _pack_inputs(inputs):
    """Host-side weight folding: everything that doesn't depend on x."""
    import ml_dtypes

    f32 = lambda n: np.asarray(inputs[n], dtype=np.float32)
    bnf = {}
    for p in ("in", "k", "v"):
        sc = f32(f"{p}_bn_gamma") / np.sqrt(f32(f"{p}_bn_var") + EPS)
        sh = f32(f"{p}_bn_beta") - f32(f"{p}_bn_mean") * sc
        bnf[p] = (sc, sh)

    def tobf(a):
        return np.ascontiguousarray(a.astype(ml_dtypes.bfloat16))

    q_w = f32("q_w")                     # [512, 640]
    qwT = np.zeros((128, NCH * 512), np.float32)
    for ch in range(NCH):
        qwT[:, 512 * ch:512 * (ch + 1)] = q_w[:, 128 * ch:128 * (ch + 1)].T

    sck, shk = bnf["k"]
    scv, shv = bnf["v"]
    kw_s = f32("k_w") * sck[None, :] * 0.125      # [64, 640]
    vw_s = f32("v_w") * scv[None, :]
    kdw = f32("k_dw_w").reshape(C, 9)
    vdw = f32("v_dw_w").reshape(C, 9)
    wtap = np.zeros((128, NCH * 9 * 128), np.float32)
    for ch in range(NCH):
        cs = slice(128 * ch, 128 * (ch + 1))
        for t in range(9):
            blk = wtap[:, 128 * (9 * ch + t):128 * (9 * ch + t + 1)]
            blk[:, 0:64] = kw_s[:, cs].T * kdw[cs, t][:, None]
            blk[:, 64:128] = vw_s[:, cs].T * vdw[cs, t][:, None]

    kvconst = np.zeros((128, 1), np.float32)
    kvconst[0:64, 0] = (kw_s @ shk)
    kvconst[64:128, 0] = (vw_s @ shv)

    out_w = f32("out_w")                 # [640, 512]
    owT = np.zeros((128, 4 * C), np.float32)
    for nv in range(4):
        owT[:, C * nv:C * (nv + 1)] = out_w[:, 128 * nv:128 * (nv + 1)].T

    sci, shi = bnf["in"]
    bnio = np.zeros((128, 2 * NCH), np.float32)
    for ch in range(NCH):
        bnio[:, ch] = sci[128 * ch:128 * (ch + 1)]
        bnio[:, NCH + ch] = shi[128 * ch:128 * (ch + 1)]

    ls = f32("ls_gamma")                 # [32]
    lsrow = np.zeros((128, 2 * S), np.float32)
    for n in range(NH):
        lsrow[32 * (n % 4), S * (n // 4):S * (n // 4 + 1)] = np.tile(ls, H)

    return {
        "p_qwT": tobf(qwT),
        "p_wtap": tobf(wtap),
        "p_owT": tobf(owT),
        "p_kvconst": np.ascontiguousarray(kvconst),
        "p_bnio": np.ascontiguousarray(bnio),
        "p_lsrow": tobf(lsrow),
    }


def make_in_maps(inputs):
    x = np.ascontiguousarray(np.asarray(inputs["x"], dtype=np.float32))
    base = _pack_inputs(inputs)
    in_maps = []
    for c in range(N_CORES):
        m = dict(base)
        m["x"] = x[c * BPC:(c + 1) * BPC]
        in_maps.append(m)
    return in_maps


_NC_CACHE = None


def kernel(**inputs):
    global _NC_CACHE
    from concourse.bass_utils import run_bass_kernel_spmd

    if _NC_CACHE is None:
        _NC_CACHE = build_nc()
    nc = _NC_CACHE

    in_maps = make_in_maps(inputs)
    res = run_bass_kernel_spmd(nc, in_maps, core_ids=list(range(N_CORES)))
    out = np.concatenate([res.results[c]["out"] for c in range(N_CORES)], axis=0)
    return out.astype(np.float32)
